# revision 2
# baseline (speedup 1.0000x reference)
"""Distributed Trainium2 kernel for nn_ContrastiveLoss (survival contrastive loss).

Strategy (8 NeuronCores, data-parallel over rows):
  host: quantile-bin rows into 4 risk groups, stable-sort rows by group,
        transpose embeddings to [D, N]; each core gets a rotated copy so
        its own 1024 rows sit at columns 0..1023 (static APs, SPMD-safe).
  device (per core): normalize columns (ssq via ones-matmul broadcast,
        sqrt, reciprocal, scale) -> z^T in f32r; for each 128-row block
        compute sim row-block via f32r matmuls (K=512 in 4 chunks),
        fused exp(10*sim-10)+row-sum on the scalar engine per 512-col
        tile; group sums = masked sums over whole tiles (groups are
        2048-aligned after sorting); subtract the exact diagonal term,
        log-ratio -> per-row loss.
  host: sum per-row losses / N.
"""
import sys

sys.path.insert(0, "/opt/trn_rl_repo")
import numpy as np

N, D, G, NCORES = 8192, 512, 4, 8
TEMP = 0.1
RPC = N // NCORES      # 1024 rows per core
RB = RPC // 128        # 8 row blocks per core
CT = 512               # column tile
NT = N // CT           # 16 column tiles
KC = D // 128          # 4 contraction chunks

_built = None


def _build():
    from concourse import bacc, tile, mybir

    nc = bacc.Bacc(None, target_bir_lowering=False)
    f32 = mybir.dt.float32
    f32r = mybir.dt.float32r
    AF = mybir.ActivationFunctionType
    AX = mybir.AxisListType

    et = nc.dram_tensor("et", [D, N], f32, kind="ExternalInput")
    sel = nc.dram_tensor("sel", [128, RB * NT], f32, kind="ExternalInput")
    dmask = nc.dram_tensor("dmask", [128, 4 * CT], f32, kind="ExternalInput")
    ones = nc.dram_tensor("ones", [128, 128], f32, kind="ExternalInput")
    pr = nc.dram_tensor("pr", [128, RB], f32, kind="ExternalOutput")

    with tile.TileContext(nc) as tc:
        with tc.tile_pool(name="zt", bufs=1) as ztp, \
             tc.tile_pool(name="cst", bufs=1) as cst, \
             tc.tile_pool(name="io", bufs=2) as io, \
             tc.tile_pool(name="eb", bufs=4) as ebp, \
             tc.tile_pool(name="sm", bufs=2) as smp, \
             tc.tile_pool(name="pp", bufs=2, space="PSUM") as ppp, \
             tc.tile_pool(name="pm", bufs=5, space="PSUM") as pmp:

            onest = cst.tile([128, 128], f32r)
            nc.sync.dma_start(onest[:], ones[:].bitcast(f32r))
            dmt = cst.tile([128, 4 * CT], f32)
            nc.sync.dma_start(dmt[:], dmask[:])
            selt = cst.tile([128, RB * NT], f32)
            nc.sync.dma_start(selt[:], sel[:])
            bias10 = cst.tile([128, 1], f32)
            nc.vector.memset(bias10[:], -10.0)
            prt = cst.tile([128, RB], f32)

            zts = [ztp.tile([128, N], f32r, tag=f"zt{k}", name=f"zt{k}")
                   for k in range(KC)]

            # ---- prep: column norms + scale -> z^T (f32r) ----
            for c in range(NT):
                cs = slice(c * CT, (c + 1) * CT)
                chunks = []
                for k in range(KC):
                    ch = io.tile([128, CT], f32, tag=f"ch{k}")
                    nc.sync.dma_start(ch[:], et[k * 128:(k + 1) * 128, cs])
                    chunks.append(ch)
                ps = ppp.tile([128, CT], f32)
                for k in range(KC):
                    sq = io.tile([128, CT], f32r, tag=f"sq{k}")
                    nc.vector.tensor_mul(sq[:], chunks[k][:], chunks[k][:])
                    nc.tensor.matmul(ps[:], onest[:], sq[:],
                                     start=(k == 0), stop=(k == KC - 1))
                st = io.tile([128, CT], f32, tag="st")
                nc.scalar.activation(st[:], ps[:], AF.Sqrt)
                rbt = io.tile([128, CT], f32, tag="rbt")
                nc.vector.reciprocal(rbt[:], st[:])
                for k in range(KC):
                    nc.vector.tensor_mul(zts[k][:, cs], chunks[k][:], rbt[:])

            # ---- main: per row-block masked logsumexp sums ----
            for r in range(RB):
                rs = slice(r * 128, (r + 1) * 128)
                ssc = smp.tile([128, NT], f32, tag="ssc")
                dval = smp.tile([128, 1], f32, tag="dval")
                for t in range(NT):
                    ts = slice(t * CT, (t + 1) * CT)
                    pm = pmp.tile([128, CT], f32)
                    for k in range(KC):
                        nc.tensor.matmul(pm[:], zts[k][:, rs], zts[k][:, ts],
                                         start=(k == 0), stop=(k == KC - 1))
                    eb = ebp.tile([128, CT], f32, tag="eb")
                    nc.scalar.activation(eb[:], pm[:], AF.Exp,
                                         bias=bias10[:], scale=1.0 / TEMP,
                                         accum_out=ssc[:, t:t + 1])
                    if t == r // 4:
                        o = r % 4
                        dt_ = ebp.tile([128, CT], f32, tag="dtmp")
                        nc.vector.tensor_mul(
                            dt_[:], eb[:], dmt[:, o * CT:(o + 1) * CT])
                        nc.vector.reduce_sum(dval[:], dt_[:], axis=AX.X)
                sall = smp.tile([128, 1], f32, tag="sall")
                nc.vector.reduce_sum(sall[:], ssc[:], axis=AX.X)
                spm = smp.tile([128, NT], f32, tag="spm")
                nc.vector.tensor_mul(spm[:], ssc[:],
                                     selt[:, r * NT:(r + 1) * NT])
                spos = smp.tile([128, 1], f32, tag="spos")
                nc.vector.reduce_sum(spos[:], spm[:], axis=AX.X)
                den = smp.tile([128, 1], f32, tag="den")
                nc.vector.tensor_sub(den[:], sall[:], dval[:])
                pos = smp.tile([128, 1], f32, tag="pos")
                nc.vector.tensor_sub(pos[:], spos[:], dval[:])
                lden = smp.tile([128, 1], f32, tag="lden")
                nc.scalar.activation(lden[:], den[:], AF.Ln)
                lpos = smp.tile([128, 1], f32, tag="lpos")
                nc.scalar.activation(lpos[:], pos[:], AF.Ln)
                nc.vector.tensor_sub(prt[:, r:r + 1], lden[:], lpos[:])

            nc.sync.dma_start(pr[:], prt[:])

    nc.finalize()
    return nc


def _get_built():
    global _built
    if _built is None:
        _built = _build()
    return _built


def _host_prep(embeddings, survival_times):
    E = np.ascontiguousarray(np.asarray(embeddings, dtype=np.float32))
    t = np.asarray(survival_times, dtype=np.float32)
    q = np.quantile(t.astype(np.float64), [0.25, 0.5, 0.75])
    rg = (t[:, None].astype(np.float64) >= q[None, :]).sum(axis=1)
    counts = np.bincount(rg, minlength=G)
    # layout assumptions: every group is a whole number of 512-col tiles
    # and every 128-row block is within one group (true for quantile bins
    # of N=8192 distinct values: 2048 per group)
    assert (counts % CT == 0).all() and (counts >= 2).all(), counts
    perm = np.argsort(rg, kind="stable")
    ET = np.ascontiguousarray(E[perm].T)  # [D, N]
    bounds = np.concatenate([[0], np.cumsum(counts)])
    gcol_global = np.searchsorted(bounds, np.arange(NT) * CT, side="right") - 1
    grow_global = np.searchsorted(bounds, np.arange(N // 128) * 128,
                                  side="right") - 1

    dmask = np.zeros((128, 4 * CT), dtype=np.float32)
    for o in range(4):
        for p in range(128):
            dmask[p, o * CT + o * 128 + p] = 1.0
    ones = np.ones((128, 128), dtype=np.float32)

    in_maps = []
    for k in range(NCORES):
        et_k = np.ascontiguousarray(np.roll(ET, -k * RPC, axis=1))
        sel_k = np.zeros((128, RB * NT), dtype=np.float32)
        for r in range(RB):
            g_row = grow_global[(k * RPC + r * 128) // 128]
            for tt in range(NT):
                gc = gcol_global[((tt * CT + k * RPC) % N) // CT]
                if gc == g_row:
                    sel_k[:, r * NT + tt] = 1.0
        in_maps.append({"et": et_k, "sel": sel_k, "dmask": dmask,
                        "ones": ones})
    return in_maps


def kernel(embeddings, survival_times, censor):
    from concourse.bass_utils import run_bass_kernel_spmd

    nc = _get_built()
    in_maps = _host_prep(embeddings, survival_times)
    res = run_bass_kernel_spmd(nc, in_maps, list(range(NCORES)))
    total = 0.0
    for i in range(NCORES):
        total += res.results[i]["pr"].astype(np.float64).sum()
    return np.float32(total / N)


# revision 11
# speedup vs baseline: 1.1462x; 1.1462x over previous
"""Distributed Trainium2 kernel for nn_ContrastiveLoss (survival contrastive loss).

Strategy (8 NeuronCores, data-parallel over rows):
  host: quantile-bin rows into 4 risk groups, stable-sort rows by group,
        transpose embeddings to [D, N]; each core gets a rotated copy so
        its own 1024 rows sit at columns 0..1023 (static APs, SPMD-safe).
  device (per core): normalize columns (ssq via ones-matmul broadcast,
        sqrt, reciprocal, scale) -> z^T in f32r; for each 128-row block
        compute sim row-block via f32r matmuls (K=512 in 4 chunks),
        fused exp(10*sim-10)+row-sum on the scalar engine per 512-col
        tile; group sums = masked sums over whole tiles (groups are
        2048-aligned after sorting); subtract the exact diagonal term,
        log-ratio -> per-row loss.
  host: sum per-row losses / N.
"""
import sys

sys.path.insert(0, "/opt/trn_rl_repo")
import numpy as np

N, D, G, NCORES = 8192, 512, 4, 8
TEMP = 0.1
RPC = N // NCORES      # 1024 rows per core
RB = RPC // 128        # 8 row blocks per core
CT = 512               # column tile
NT = N // CT           # 16 column tiles
KC = D // 128          # 4 contraction chunks

_built = None


def _build():
    from concourse import bacc, tile, mybir

    nc = bacc.Bacc(None, target_bir_lowering=False)
    f32 = mybir.dt.float32
    f32r = mybir.dt.float32r
    AF = mybir.ActivationFunctionType
    AX = mybir.AxisListType

    et = nc.dram_tensor("et", [D, N], f32, kind="ExternalInput")
    sel = nc.dram_tensor("sel", [128, RB * NT], f32, kind="ExternalInput")
    dmask = nc.dram_tensor("dmask", [128, 4 * CT], f32, kind="ExternalInput")
    ones = nc.dram_tensor("ones", [128, 128], f32, kind="ExternalInput")
    pr = nc.dram_tensor("pr", [128, RB], f32, kind="ExternalOutput")

    with tile.TileContext(nc) as tc:
        with tc.tile_pool(name="zt", bufs=1) as ztp, \
             tc.tile_pool(name="cst", bufs=1) as cst, \
             tc.tile_pool(name="io", bufs=2) as io, \
             tc.tile_pool(name="eb", bufs=6) as ebp, \
             tc.tile_pool(name="sm", bufs=3) as smp, \
             tc.tile_pool(name="pp", bufs=2, space="PSUM") as ppp, \
             tc.tile_pool(name="pm", bufs=6, space="PSUM") as pmp:

            # preload the exp+ln activation table once; all ACT funcs used
            # below (Exp, Ln) live in set 6 = natural_log_exp_and_others,
            # so the act-table fixpoint pass inserts no further reloads
            nc.scalar.add_instruction(
                mybir.InstLoadActFuncSet(
                    name=nc.get_next_instruction_name(),
                    act_func_set_id=6, ins=[], outs=[]))

            onest = cst.tile([128, 128], f32r)
            nc.sync.dma_start(onest[:], ones[:].bitcast(f32r))
            dmt = cst.tile([128, 4 * CT], f32)
            nc.sync.dma_start(dmt[:], dmask[:])
            selt = cst.tile([128, RB * NT], f32)
            nc.sync.dma_start(selt[:], sel[:])
            bias10 = cst.tile([128, 1], f32)
            nc.vector.memset(bias10[:], -10.0)
            prt = cst.tile([128, RB], f32)

            zts = [ztp.tile([128, N], f32r, tag=f"zt{k}", name=f"zt{k}")
                   for k in range(KC)]

            # ---- prep: column norms + scale -> z^T (f32r) ----
            for c in range(NT):
                cs = slice(c * CT, (c + 1) * CT)
                chunks = []
                for k in range(KC):
                    ch = io.tile([128, CT], f32, tag=f"ch{k}")
                    nc.sync.dma_start(ch[:], et[k * 128:(k + 1) * 128, cs])
                    chunks.append(ch)
                ps = ppp.tile([128, CT], f32)
                for k in range(KC):
                    sq = io.tile([128, CT], f32r, tag=f"sq{k}")
                    # split squares across DVE and ACT: prep is DVE-bound
                    # while ACT idles (Square is in table set 6 -> no reload)
                    if k < 2:
                        nc.vector.tensor_mul(sq[:], chunks[k][:], chunks[k][:])
                    else:
                        nc.scalar.activation(sq[:], chunks[k][:], AF.Square)
                    nc.tensor.matmul(ps[:], onest[:], sq[:],
                                     start=(k == 0), stop=(k == KC - 1))
                # rsqrt(ssq) = exp(-0.5*ln(ssq)): keeps every ACT op inside
                # the natural_log_exp_and_others table set (no table reloads)
                st = io.tile([128, CT], f32, tag="st")
                nc.scalar.activation(st[:], ps[:], AF.Ln)
                rbt = io.tile([128, CT], f32, tag="rbt")
                nc.scalar.activation(rbt[:], st[:], AF.Exp, scale=-0.5)
                for k in range(KC):
                    nc.vector.tensor_mul(zts[k][:, cs], chunks[k][:], rbt[:])

            # ---- main: per row-block masked logsumexp sums ----
            dens = cst.tile([128, RB], f32)
            poss = cst.tile([128, RB], f32)
            for r in range(RB):
                rs = slice(r * 128, (r + 1) * 128)
                ssc = smp.tile([128, NT], f32, tag="ssc")
                dval = smp.tile([128, 1], f32, tag="dval")
                for t in range(NT):
                    ts = slice(t * CT, (t + 1) * CT)
                    pm = pmp.tile([128, CT], f32)
                    for k in range(KC):
                        nc.tensor.matmul(pm[:], zts[k][:, rs], zts[k][:, ts],
                                         start=(k == 0), stop=(k == KC - 1))
                    eb = ebp.tile([128, CT], f32, tag="eb")
                    nc.scalar.activation(eb[:], pm[:], AF.Exp,
                                         bias=bias10[:], scale=1.0 / TEMP,
                                         accum_out=ssc[:, t:t + 1])
                    if t == r // 4:
                        o = r % 4
                        dt_ = ebp.tile([128, CT], f32, tag="dtmp")
                        nc.vector.tensor_mul(
                            dt_[:], eb[:], dmt[:, o * CT:(o + 1) * CT])
                        nc.vector.reduce_sum(dval[:], dt_[:], axis=AX.X)
                sall = smp.tile([128, 1], f32, tag="sall")
                nc.vector.reduce_sum(sall[:], ssc[:], axis=AX.X)
                spm = smp.tile([128, NT], f32, tag="spm")
                nc.vector.tensor_mul(spm[:], ssc[:],
                                     selt[:, r * NT:(r + 1) * NT])
                spos = smp.tile([128, 1], f32, tag="spos")
                nc.vector.reduce_sum(spos[:], spm[:], axis=AX.X)
                nc.vector.tensor_sub(dens[:, r:r + 1], sall[:], dval[:])
                nc.vector.tensor_sub(poss[:, r:r + 1], spos[:], dval[:])

            # batched tail: 2 Ln + 1 sub for all row blocks
            ldens = cst.tile([128, RB], f32)
            nc.scalar.activation(ldens[:], dens[:], AF.Ln)
            lposs = cst.tile([128, RB], f32)
            nc.scalar.activation(lposs[:], poss[:], AF.Ln)
            nc.vector.tensor_sub(prt[:], ldens[:], lposs[:])

            nc.sync.dma_start(pr[:], prt[:])

    nc.finalize()
    return nc


def _get_built():
    global _built
    if _built is None:
        _built = _build()
    return _built


def _host_prep(embeddings, survival_times):
    E = np.ascontiguousarray(np.asarray(embeddings, dtype=np.float32))
    t = np.asarray(survival_times, dtype=np.float32)
    q = np.quantile(t.astype(np.float64), [0.25, 0.5, 0.75])
    rg = (t[:, None].astype(np.float64) >= q[None, :]).sum(axis=1)
    counts = np.bincount(rg, minlength=G)
    # layout assumptions: every group is a whole number of 512-col tiles
    # and every 128-row block is within one group (true for quantile bins
    # of N=8192 distinct values: 2048 per group)
    assert (counts % CT == 0).all() and (counts >= 2).all(), counts
    perm = np.argsort(rg, kind="stable")
    ET = np.ascontiguousarray(E[perm].T)  # [D, N]
    bounds = np.concatenate([[0], np.cumsum(counts)])
    gcol_global = np.searchsorted(bounds, np.arange(NT) * CT, side="right") - 1
    grow_global = np.searchsorted(bounds, np.arange(N // 128) * 128,
                                  side="right") - 1

    dmask = np.zeros((128, 4 * CT), dtype=np.float32)
    for o in range(4):
        for p in range(128):
            dmask[p, o * CT + o * 128 + p] = 1.0
    ones = np.ones((128, 128), dtype=np.float32)

    in_maps = []
    for k in range(NCORES):
        et_k = np.ascontiguousarray(np.roll(ET, -k * RPC, axis=1))
        sel_k = np.zeros((128, RB * NT), dtype=np.float32)
        for r in range(RB):
            g_row = grow_global[(k * RPC + r * 128) // 128]
            for tt in range(NT):
                gc = gcol_global[((tt * CT + k * RPC) % N) // CT]
                if gc == g_row:
                    sel_k[:, r * NT + tt] = 1.0
        in_maps.append({"et": et_k, "sel": sel_k, "dmask": dmask,
                        "ones": ones})
    return in_maps


def kernel(embeddings, survival_times, censor):
    from concourse.bass_utils import run_bass_kernel_spmd

    nc = _get_built()
    in_maps = _host_prep(embeddings, survival_times)
    res = run_bass_kernel_spmd(nc, in_maps, list(range(NCORES)))
    total = 0.0
    for i in range(NCORES):
        total += res.results[i]["pr"].astype(np.float64).sum()
    return np.float32(total / N)
